# revision 9
# baseline (speedup 1.0000x reference)
"""Trainium2 Bass kernel for nn_Classification2 (histogram_binning).

matrix[x, y] = -mean((clip1[y] - clip2[x])**2) * 1e13 over D = 3*224*224
             = -(SCALE/D) * (||a_x||^2 + ||b_y||^2 - 2 a_x.b_y)
output[k]    = mean of matrix over diagonals y - x = k - 64, k in [0, 129)

Strategy: data-parallel over D across 8 NeuronCores. The host pre-transposes
each core's D-shard into an fp8(e4m3) [p=128, f=147, 256] tensor whose
columns are [B_f | A_f], so the device DMA is one contiguous stream and the
PE contracts over the partition axis with no on-chip transposes. fp8 halves
the HBM traffic vs bf16 (this kernel is memory-bound) and DoubleRow perf
mode contracts two 128-deep k-chunks per instruction (~157 TF/s measured),
so the PE keeps pace with the DMA stream. Per k-chunk-pair the PE runs one
N=256 DoubleRow matmul (lhsT=A pair, rhs=[B|A] pair) accumulating
[gram | A-gram] and one N=128 DoubleRow matmul (lhsT=rhs=B pair) for
B-gram; the odd 147th chunk uses two plain fp8 matmuls. The tail evacuates
everything into one [128, 384] SBUF tile = [scaled gram | A-gram | B-gram]
and dumps it with a single DMA; the host sums the 8 per-core dumps, takes
diagonal sums of the gram and the two norm diagonals, and finishes the
O(S^2) binning.

fp8 is safe here: the result is a mean over >=64 diagonal entries of sums
of 150528 products; host emulation of the exact quantize+accumulate gives
7.2e-4 relative error vs the 2e-2 gate.

Input DMAs alternate the two HWDGE queues (sync/scalar) with ramped chunk
sizes. Completion order matches program order (measured: a 3rd queue or
coarse chunks make chunk completions arrive out of order and stall the PE
for ~5us mid-stream), chunks are small at the start so the PE spins up
~1.5us after 'main' and small at the end so the PE + evacuation finish
right behind the last descriptor.
"""

import sys

sys.path.insert(0, "/opt/trn_rl_repo")

import numpy as np

S = 128
D = 150528  # 3*224*224
N_CORES = 8
DC = D // N_CORES  # 18816 d-values per core
F = DC // S  # 147 contraction chunks of K=128
# ramped chunk sizes (f units), alternating the two HWDGE queues: small
# first for fast PE start, large in the middle to amortize the ~0.65us
# per-issue descriptor-generation cost, small at the end so the PE tail is
# short. All chunk starts are even so DoubleRow pairs never straddle
# chunks; the final 1-wide chunk is the odd k-chunk (plain fp8 matmuls).
CHUNK_F = [4, 8, 8, 10, 12, 14, 16, 16, 16, 14, 12, 10, 6, 1]
# PE clock warmup: the Tensor engine ramps to full clock only after ~3us of
# continuous execution. Dummy matmuls on a zeroed scratch tile fill the
# ~3us between 'main' and the first chunk's arrival so the real matmuls run
# at full speed from the start.
N_WARMUP = 15
assert sum(CHUNK_F) == F
assert all(s % 2 == 0 for s in np.cumsum([0] + CHUNK_F[:-1]))
SCALE = 1.0e13
EVAC_SCALE = 2.0 * SCALE / D  # psum gram + bias path

_NC_CACHE = {}


def _build():
    import concourse.bacc as bacc
    import concourse.mybir as mybir
    import concourse.tile as tile

    f32 = mybir.dt.float32
    fp8 = mybir.dt.float8e4
    DR = mybir.MatmulPerfMode.DoubleRow

    nc = bacc.Bacc(num_devices=N_CORES)

    ba_in = nc.dram_tensor("ba", [S, F, 256], fp8, kind="ExternalInput")
    # out rows: [scaled gram (128) | A-gram (128) | B-gram (128)]
    out_t = nc.dram_tensor("out", [S, 3 * S], f32, kind="ExternalOutput")

    with tile.TileContext(nc) as tc:
        with (
            tc.tile_pool(name="ba_pool", bufs=1) as ba_pool,
            tc.tile_pool(name="misc", bufs=1) as misc,
            tc.tile_pool(name="psum", bufs=1, space="PSUM") as psum,
        ):
            # input chunk DMAs first, alternating the two HWDGE issue queues
            queues = [nc.sync, nc.scalar]
            ba_tiles = []
            f0 = 0
            for ci, nf in enumerate(CHUNK_F):
                t = ba_pool.tile([S, nf, 256], fp8, tag=f"ba{ci}")
                eng = queues[ci % len(queues)]
                eng.dma_start(out=t[:, :, :], in_=ba_in[:, f0 : f0 + nf, :])
                ba_tiles.append((t, f0, nf))
                f0 += nf

            ps_wide = psum.tile([S, 256], f32, tag="ps_wide")
            ps_bg = psum.tile([S, S], f32, tag="ps_bg")

            # PE warmup while the first chunks stream in
            wt = misc.tile([S, 256], fp8, tag="wt")
            nc.vector.memset(wt[:, :], 0.0)
            ps_dummy = psum.tile([S, 256], f32, tag="ps_dummy")
            for _ in range(N_WARMUP):
                nc.tensor.matmul(
                    ps_dummy[:, :], wt[:, S:256], wt[:, :], start=True, stop=True
                )

            first = True
            for t, f0, nf in ba_tiles:
                if nf == 1:
                    # odd tail k-chunk: plain fp8 matmuls, and the global stop
                    nc.tensor.matmul(
                        ps_wide[:, :],
                        t[:, 0:1, S:256],
                        t[:, 0:1, :],
                        start=False,
                        stop=True,
                    )
                    nc.tensor.matmul(
                        ps_bg[:, :],
                        t[:, 0:1, 0:S],
                        t[:, 0:1, 0:S],
                        start=False,
                        stop=True,
                    )
                    continue
                for j in range(0, nf, 2):
                    nc.tensor.matmul(
                        ps_wide[:, :],
                        t[:, j : j + 2, S:256],
                        t[:, j : j + 2, :],
                        start=first,
                        stop=False,
                        perf_mode=DR,
                    )
                    nc.tensor.matmul(
                        ps_bg[:, :],
                        t[:, j : j + 2, 0:S],
                        t[:, j : j + 2, 0:S],
                        start=first,
                        stop=False,
                        perf_mode=DR,
                    )
                    first = False

            # evacuate into one [128, 384] tile (scaled gram on ACT, raw
            # A/B-gram on DVE — parallel engines), then dump via two DMAs on
            # separate queues so each issues as soon as its half is written
            sb_out = misc.tile([S, 3 * S], f32, tag="sb_out")
            nc.scalar.mul(sb_out[:, 0:S], ps_wide[:, 0:S], EVAC_SCALE)
            nc.vector.tensor_copy(sb_out[:, S : 2 * S], ps_wide[:, S:256])
            nc.vector.tensor_copy(sb_out[:, 2 * S : 3 * S], ps_bg[:, :])
            nc.sync.dma_start(out=out_t[:, 0:S], in_=sb_out[:, 0:S])
            nc.scalar.dma_start(
                out=out_t[:, S : 3 * S], in_=sb_out[:, S : 3 * S]
            )

    nc.finalize()
    return nc


def _get_nc():
    if "nc" not in _NC_CACHE:
        _NC_CACHE["nc"] = _build()
    return _NC_CACHE["nc"]


def _shards(clip1: np.ndarray, clip2: np.ndarray):
    """Per-core fp8 [S, F, 256] tensors: cols [B_f | A_f] per f, where
    value (p, f, x) = clip[x, d0 + f*128 + p]."""
    import ml_dtypes

    fp8 = ml_dtypes.float8_e4m3
    c1 = np.ascontiguousarray(np.asarray(clip1), dtype=np.float32).reshape(S, D)
    c2 = np.ascontiguousarray(np.asarray(clip2), dtype=np.float32).reshape(S, D)
    maps = []
    for c in range(N_CORES):
        sl = slice(c * DC, (c + 1) * DC)
        bt = c1[:, sl].reshape(S, F, S).transpose(2, 1, 0)  # [p, f, y] moving
        at = c2[:, sl].reshape(S, F, S).transpose(2, 1, 0)  # [p, f, x] stationary
        ba = np.empty((S, F, 256), dtype=fp8)
        ba[:, :, 0:S] = bt.astype(fp8)
        ba[:, :, S:256] = at.astype(fp8)
        maps.append({"ba": ba})
    return maps


def _combine(results) -> np.ndarray:
    total = np.zeros((S, 3 * S), dtype=np.float64)
    for r in results:
        total += np.asarray(r["out"], dtype=np.float64)
    g = total[:, 0:S]  # (2*SCALE/D) * gram, gram[x, y] = a_x . b_y
    sq_a = np.diag(total[:, S : 2 * S])
    sq_b = np.diag(total[:, 2 * S : 3 * S])
    pa = np.concatenate([[0.0], np.cumsum(sq_a)])
    pb = np.concatenate([[0.0], np.cumsum(sq_b)])
    out = np.empty(S + 1, dtype=np.float64)
    for i in range(S + 1):
        o = i - 64  # diagonal offset y - x
        x0, x1 = max(0, -o), S - max(0, o)  # valid x in [x0, x1)
        wa = pa[x1] - pa[x0]
        wb = pb[x1 + o] - pb[x0 + o]
        out[i] = (np.trace(g, offset=o) - (SCALE / D) * (wa + wb)) / (x1 - x0)
    return out.astype(np.float32)


def kernel(clip1: np.ndarray, clip2: np.ndarray, **_ignored) -> np.ndarray:
    from concourse.bass_utils import run_bass_kernel_spmd

    in_maps = _shards(clip1, clip2)
    nc = _get_nc()
    res = run_bass_kernel_spmd(nc, in_maps, core_ids=list(range(N_CORES)))
    return _combine(res.results)
